# revision 4
# baseline (speedup 1.0000x reference)
"""GCN cascade layer (3 parallel GCNConv + 1 linear head) on 8 Trainium2 cores.

Reformulation (per edge set i):
    norm_e = dinv[row_e] * w_e * dinv[col_e]        (host, f32)
    s[c]   = sum_{e->c} norm_e * x[row_e]            (device: gather + one-hot matmul)
    out[c] = relu(W_i s[c] + b_i)                    (device: per-tile matmul)

W commutes with the edge aggregation, so no dense h = x @ W.T phase is needed.
The scatter accumulates TRANSPOSED: psT[f, c] += src^T @ bw where
src = gathered x rows [slot, f] (lhsT) and bw[slot, c] = one-hot(col) * norm
(rhs), chained over K_u chunks per 128-col tile in PSUM. Then
out^T[g, c] = W_i @ psT (+ b_i via rank-1 matmul), relu, DMA out
feature-major; the host transposes at assembly.

Distribution: output node tiles sharded over 8 cores (TPC=ceil(N/8/128)
tiles each); x replicated (gather source in DRAM, fp16, split into two
int16-addressable overlapping windows lo=[0,32768) / hi=[N2-32768,N2));
flexible rows are assigned lo/hi per (core,set,tile) to balance chunk counts.
"""

import sys

sys.path.insert(0, "/opt/trn_rl_repo")

import math
from dataclasses import dataclass

import numpy as np

import concourse.bass as bass
import concourse.bacc as bacc
import concourse.mybir as mybir
from concourse import tile

P = 128
CORES = 8
GATHER_GROUP = 7   # node tiles per dma_gather call
PAD_COL = 999.0    # colloc sentinel for padded slots (never equals iota)

f16 = mybir.dt.float16
f32 = mybir.dt.float32
i16 = mybir.dt.int16

LAST_RESULTS = None
TRACE = False


@dataclass
class Cfg:
    N: int
    E: int
    A: int
    TPC: int
    NT: int
    N2: int
    OWN: int
    LO_CAP: int      # rows covered by the lo window [0, LO_CAP)
    HI0: int         # hi window start (rows [HI0, N2))
    K_lo: int = 0
    K_hi: int = 0
    K_u: int = 0
    n_groups: int = 0
    group_tiles: tuple = ()
    group_starts: tuple = ()
    colmap: object = None


def _make_cfg(N, E, A):
    TPC = math.ceil(N / (CORES * P))
    NT = CORES * TPC
    N2 = NT * P
    LO_CAP = min(32768, N2)
    HI0 = max(0, N2 - 32768)
    cfg = Cfg(N=N, E=E, A=A, TPC=TPC, NT=NT, N2=N2, OWN=TPC * P,
              LO_CAP=LO_CAP, HI0=HI0)
    g = min(GATHER_GROUP, TPC)
    if TPC > 2 * g:
        # small first group (earlier first gather) and small last group
        # (shorter post-gather tail)
        rest = TPC - 2
        k = rest // g
        rem = rest - k * g
        sizes = [2] + [g] * k + ([rem] if rem else [])
        if rem and rem < 3 and k:
            sizes = [2] + [g] * (k - 1) + [g + rem - 3, 3]
        cfg.group_tiles = tuple(sizes)
    else:
        cfg.group_tiles = tuple(
            min(g, TPC - i * g) for i in range(math.ceil(TPC / g)))
    cfg.n_groups = len(cfg.group_tiles)
    starts = [0]
    for t in cfg.group_tiles[:-1]:
        starts.append(starts[-1] + t)
    cfg.group_starts = tuple(starts)
    return cfg


def _prep(cfg, x, edge_index, edge_attr, lin_w, lin_b, conv_w, conv_b):
    A, N, E, N2 = cfg.A, cfg.N, cfg.E, cfg.N2
    TPC, OWN = cfg.TPC, cfg.OWN

    row = edge_index[:, 0, :].astype(np.int64)    # [A, E]
    col0 = edge_index[:, 1, :].astype(np.int64)
    w = edge_attr.astype(np.float64)

    # ---- normalization (host, original col ids) ----
    norm = np.empty((A, E), np.float32)
    for i in range(A):
        deg = np.bincount(col0[i], weights=w[i], minlength=N2)
        dinv = np.where(deg > 0, 1.0 / np.sqrt(np.maximum(deg, 1e-30)), 0.0)
        norm[i] = (dinv[row[i]] * w[i] * dinv[col0[i]]).astype(np.float32)

    # ---- column rebalance: permute cols across tiles so that every
    # (tile, set) edge count fits the minimal chunk budget. colmap[c] = new id.
    import time as _time
    NTb = cfg.NT
    d = np.zeros((A, N2), np.int64)
    for i in range(A):
        d[i] = np.bincount(col0[i], minlength=N2)
    dtot = d.sum(0)
    order_c = np.argsort(-dtot, kind="stable")
    # load-feedback snake deal
    loads = np.zeros((NTb, A), np.float64)
    bin_of = np.empty(N2, np.int64)
    for r in range(N2 // NTb):
        seg = order_c[r * NTb:(r + 1) * NTb]
        border = np.argsort(loads.max(1) + 0.25 * loads.sum(1), kind="stable")
        bin_of[seg] = border
        loads[border] += d[:, seg].T
    pl = np.zeros((NTb, A), np.int64)
    for i in range(A):
        pl[:, i] = np.bincount(bin_of, weights=d[i], minlength=NTb).astype(np.int64)
    # swap-repair toward cap = next lower multiple of P when within reach
    CAP = max(P, int(math.ceil(max(pl.max(), 1) / P) - 1) * P)
    if pl.mean() < CAP and np.maximum(pl - CAP, 0).sum() > 0:
        cols_of = [list(np.where(bin_of == b)[0]) for b in range(NTb)]
        t0 = _time.time()
        rng_r = np.random.default_rng(12345)
        it = 0
        while np.maximum(pl - CAP, 0).sum() > 0 and it < 5000 \
                and _time.time() - t0 < 45:
            it += 1
            b = int(np.argmax(np.maximum(pl - CAP, 0).sum(1)))
            slack = (CAP - pl).clip(0, None)
            need = np.maximum(pl[b] - CAP, 0)
            cand = np.argsort(-(slack * (need > 0)).sum(1))[:12]
            best = None
            cb_arr = np.array(cols_of[b])
            db = d[:, cb_arr]
            base_b = np.maximum(pl[b] - CAP, 0).sum()
            for b2 in cand:
                if b2 == b:
                    continue
                cb2_arr = np.array(cols_of[b2])
                db2 = d[:, cb2_arr]
                delta = db[:, :, None] - db2[:, None, :]
                nb_new = pl[b][:, None, None] - delta
                n2_new = pl[b2][:, None, None] + delta
                score = (np.maximum(nb_new - CAP, 0).sum(0)
                         + np.maximum(n2_new - CAP, 0).sum(0))
                ix = np.unravel_index(np.argmin(score), score.shape)
                base = base_b + np.maximum(pl[b2] - CAP, 0).sum()
                gain = score[ix] - base
                if gain < 0 and (best is None or gain < best[0]):
                    best = (gain, b2, ix[0], ix[1])
            if best is None:
                b2 = int(rng_r.integers(0, NTb))
                if b2 == b:
                    continue
                best = (0, b2, int(rng_r.integers(0, len(cols_of[b]))),
                        int(rng_r.integers(0, len(cols_of[b2]))))
            _, b2, i1, i2 = best
            c1, c2 = cols_of[b][i1], cols_of[b2][i2]
            cols_of[b][i1], cols_of[b2][i2] = c2, c1
            pl[b] += d[:, c2] - d[:, c1]
            pl[b2] += d[:, c1] - d[:, c2]
        for b in range(NTb):
            bin_of[np.array(cols_of[b])] = b
    # slots within bin: stable order
    colmap = np.empty(N2, np.int64)
    for b in range(NTb):
        cb = np.where(bin_of == b)[0]
        colmap[cb] = b * P + np.arange(len(cb))
    cfg.colmap = colmap
    col = colmap[col0]                                 # [A, E] permuted cols

    # ---- lo/hi stream assignment, balanced per (set, core, tile) ----
    # bucket id per edge: (i*CORES + core)*TPC + tloc
    core = col // OWN                                  # [A, E]
    tloc = (col % OWN) // P
    cint = (col % P).astype(np.float32)
    bucket = (np.arange(A)[:, None] * CORES + core) * TPC + tloc   # [A, E]
    NB = A * CORES * TPC

    use_hi = cfg.N2 > cfg.LO_CAP
    must_hi = row >= cfg.LO_CAP
    flex = (row >= cfg.HI0) & ~must_hi if use_hi else np.zeros_like(must_hi)

    bucket_f = bucket.ravel()
    must_hi_f = must_hi.ravel()
    flex_f = flex.ravel()
    n_tot = np.bincount(bucket_f, minlength=NB)
    n_mhi = np.bincount(bucket_f[must_hi_f], minlength=NB)
    n_flex = np.bincount(bucket_f[flex_f], minlength=NB)
    n_mlo = n_tot - n_mhi - n_flex
    # asymmetric split: cap the hi stream at K_tot//2 chunks so
    # K_lo + K_hi == ceil(max_tot/128) (minimal total chunk count)
    K_tot = max(1, int(math.ceil(n_tot.max() / P)))
    hi_cap = (K_tot // 2) * P if use_hi else 0
    lo_cnt = np.clip(n_tot - hi_cap, n_mlo, n_mlo + n_flex)
    flex_to_lo = lo_cnt - n_mlo

    # stable sort edges by (bucket, class) where class: 0=must_lo,1=flex,2=must_hi
    cls_f = np.zeros(A * E, np.int8)
    cls_f[flex_f] = 1
    cls_f[must_hi_f] = 2
    order = np.lexsort((cls_f, bucket_f))
    # rank within (bucket, class) for flex edges to split them
    key = bucket_f[order] * 4 + cls_f[order]
    change = np.r_[True, key[1:] != key[:-1]]
    starts = np.where(change, np.arange(A * E), 0)
    starts = np.maximum.accumulate(starts)
    rank_in_cls = np.arange(A * E) - starts
    is_lo_sorted = (cls_f[order] == 0) | (
        (cls_f[order] == 1) & (rank_in_cls < flex_to_lo[bucket_f[order]]))
    is_lo = np.empty(A * E, np.bool_)
    is_lo[order] = is_lo_sorted
    is_lo = is_lo.reshape(A, E)

    # ---- per-stream rank within bucket ----
    strm = (~is_lo).astype(np.int8).ravel()            # 0=lo, 1=hi
    order2 = np.lexsort((row.ravel(), strm, bucket_f))  # sort by row for locality
    key2 = bucket_f[order2] * 2 + strm[order2]
    change2 = np.r_[True, key2[1:] != key2[:-1]]
    starts2 = np.where(change2, np.arange(A * E), 0)
    starts2 = np.maximum.accumulate(starts2)
    rank2 = np.arange(A * E) - starts2                 # rank within (bucket, stream)
    rank = np.empty(A * E, np.int64)
    rank[order2] = rank2
    rank = rank.reshape(A, E)

    n_lo_b = np.bincount(bucket_f[~strm.astype(bool)], minlength=NB)
    n_hi_b = np.bincount(bucket_f[strm.astype(bool)], minlength=NB)
    K_lo = max(1, int(math.ceil(n_lo_b.max() / P)))
    K_hi = int(math.ceil(n_hi_b.max() / P)) if use_hi else 0
    cfg.K_lo, cfg.K_hi, cfg.K_u = K_lo, K_hi, K_lo + K_hi

    # ---- metadata arrays ----
    CH = TPC * cfg.K_u
    colloc = np.full((CORES, A, P, CH), PAD_COL, np.float16)
    wchunk = np.zeros((CORES, A, P, CH), np.float16)
    # gather idx streams (per core, set): [16, TPC*K*8] -> tiled to 128 parts
    gl_len = TPC * K_lo * P
    gh_len = max(TPC * K_hi * P, 16)
    gidx_lo = np.zeros((CORES, A, 16, gl_len // 16), np.int16)
    gidx_hi = np.zeros((CORES, A, 16, gh_len // 16), np.int16)

    kk = rank // P
    jj = (rank % P).astype(np.int64)
    lo_m = is_lo
    k_in_tile = np.where(lo_m, kk, K_lo + kk)
    gc = tloc * cfg.K_u + k_in_tile                    # chunk column [A, E]
    ci = np.arange(A)[:, None].repeat(E, 1)
    colloc[core, ci, jj, gc] = cint
    wchunk[core, ci, jj, gc] = norm

    gi = np.where(lo_m, row, row - cfg.HI0).astype(np.int64)
    Kstr = np.where(lo_m, K_lo, K_hi)
    q = tloc * (Kstr * P) + rank                       # stream position
    lo_sel = lo_m
    gidx_lo[core[lo_sel], ci[lo_sel], q[lo_sel] % 16, q[lo_sel] // 16] = \
        gi[lo_sel].astype(np.int16)
    if cfg.K_hi > 0:
        hi_sel = ~lo_m
        gidx_hi[core[hi_sel], ci[hi_sel], q[hi_sel] % 16, q[hi_sel] // 16] = \
            gi[hi_sel].astype(np.int16)

    # ---- dense inputs ----
    xpad = np.zeros((N2 + 8, P), np.float32)
    xpad[:N] = x
    xrows = xpad.astype(np.float16)
    xlo = np.ascontiguousarray(xrows[:cfg.LO_CAP])
    xhi = np.ascontiguousarray(xrows[cfg.HI0:cfg.HI0 + min(32768, N2)])
    xT = np.ascontiguousarray(xrows[:N2].T)            # [P, N2] for hs0

    WT = np.ascontiguousarray(conv_w.transpose(0, 2, 1)).astype(np.float16)
    b_rows = conv_b.reshape(A, 1, P).astype(np.float16)
    linWT = np.ascontiguousarray(lin_w.T).astype(np.float16)
    lin_b_row = lin_b.reshape(1, P).astype(np.float16)
    ones_row = np.ones((1, P), np.float16)
    iota_wide = np.tile(np.arange(P, dtype=np.float16), (P, cfg.K_u))

    in_maps = []
    for k in range(CORES):
        m = dict(
            xlo=xlo, xhi=xhi,
            xT_own=np.ascontiguousarray(xT[:, k * OWN:(k + 1) * OWN]),
            WT=WT, b_rows=b_rows, linWT=linWT, lin_b_row=lin_b_row,
            ones_row=ones_row, iota_wide=iota_wide,
            colloc=colloc[k], wchunk=wchunk[k],
            gidx_lo=np.tile(gidx_lo[k], (1, 8, 1)),
        )
        if cfg.K_hi > 0:
            m["gidx_hi"] = np.tile(gidx_hi[k], (1, 8, 1))
        in_maps.append(m)
    return in_maps


def _build(cfg):
    nc = bacc.Bacc()
    A, TPC = cfg.A, cfg.TPC
    K_lo, K_hi, K_u = cfg.K_lo, cfg.K_hi, cfg.K_u
    CH = TPC * K_u
    Alu = mybir.AluOpType
    Act = mybir.ActivationFunctionType

    xlo_d = nc.dram_tensor("xlo", [cfg.LO_CAP, P], f16, kind="ExternalInput")
    xhi_d = nc.dram_tensor("xhi", [min(32768, cfg.N2), P], f16,
                           kind="ExternalInput")
    xT_own = nc.dram_tensor("xT_own", [P, cfg.OWN], f16, kind="ExternalInput")
    WT_d = nc.dram_tensor("WT", [A, P, P], f16, kind="ExternalInput")
    b_rows = nc.dram_tensor("b_rows", [A, 1, P], f16, kind="ExternalInput")
    linWT_d = nc.dram_tensor("linWT", [P, P], f16, kind="ExternalInput")
    lin_b_row = nc.dram_tensor("lin_b_row", [1, P], f16, kind="ExternalInput")
    ones_row = nc.dram_tensor("ones_row", [1, P], f16, kind="ExternalInput")
    iota_wide = nc.dram_tensor("iota_wide", [P, K_u * P], f16, kind="ExternalInput")
    colloc_d = nc.dram_tensor("colloc", [A, P, CH], f16, kind="ExternalInput")
    wchunk_d = nc.dram_tensor("wchunk", [A, P, CH], f16, kind="ExternalInput")
    gl_len = TPC * K_lo * P
    gh_len = max(TPC * K_hi * P, 16)
    gidx_lo_d = nc.dram_tensor("gidx_lo", [A, P, gl_len // 16], i16,
                               kind="ExternalInput")
    gidx_hi_d = (nc.dram_tensor("gidx_hi", [A, P, gh_len // 16], i16,
                                kind="ExternalInput") if K_hi > 0 else None)

    hs0 = nc.dram_tensor("hs0", [cfg.OWN, P], f16, kind="ExternalOutput")
    outsT = [nc.dram_tensor(f"outT{i}", [P, cfg.OWN], f16, kind="ExternalOutput")
             for i in range(A)]

    GT = min(GATHER_GROUP, TPC)

    with tile.TileContext(nc) as tc:
        with (
            tc.tile_pool(name="const", bufs=1) as cpool,
            tc.tile_pool(name="meta", bufs=1) as mpool,
            tc.tile_pool(name="xw", bufs=3) as xpool,
            tc.tile_pool(name="glo", bufs=3) as glo_pool,
            tc.tile_pool(name="ghi", bufs=3) as ghi_pool,
            tc.tile_pool(name="gix", bufs=4) as gix_pool,
            tc.tile_pool(name="bw", bufs=6) as bwpool,
            tc.tile_pool(name="st", bufs=4) as spool,
            tc.tile_pool(name="outst", bufs=4) as opool,
            tc.tile_pool(name="psa", bufs=4, space="PSUM") as psa,
            tc.tile_pool(name="psb", bufs=2, space="PSUM") as psb,
        ):
            # ---- gather issue helper (idempotent per (set, group)) ----
            gather_bufs = {}

            def issue_gathers(i, g):
                if (i, g) in gather_bufs:
                    return gather_bufs[(i, g)]
                tg = cfg.group_tiles[g]
                t0 = cfg.group_starts[g]
                L = tg * K_lo * P
                glo = glo_pool.tile([P, GT * K_lo, P], f16, tag="glo")
                gixt = gix_pool.tile([P, GT * K_lo * 8], i16, tag="gixlo")
                off = t0 * K_lo * 8
                nc.sync.dma_start(out=gixt[:, :L // 16],
                                  in_=gidx_lo_d[i, :, off:off + L // 16])
                nc.gpsimd.dma_gather(
                    out_ap=glo[:, :tg * K_lo, :], in_ap=xlo_d[:, :],
                    idxs_ap=gixt[:, :L // 16],
                    num_idxs=L, num_idxs_reg=L, elem_size=P,
                    single_packet=False)
                ghi = None
                if K_hi > 0:
                    Lh = tg * K_hi * P
                    ghi = ghi_pool.tile([P, GT * K_hi, P], f16, tag="ghi")
                    gixh = gix_pool.tile([P, GT * K_hi * 8], i16, tag="gixhi")
                    offh = t0 * K_hi * 8
                    nc.sync.dma_start(out=gixh[:, :Lh // 16],
                                      in_=gidx_hi_d[i, :, offh:offh + Lh // 16])
                    nc.gpsimd.dma_gather(
                        out_ap=ghi[:, :tg * K_hi, :], in_ap=xhi_d[:, :],
                        idxs_ap=gixh[:, :Lh // 16],
                        num_idxs=Lh, num_idxs_reg=Lh, elem_size=P,
                        single_packet=False)
                gather_bufs[(i, g)] = (glo, ghi)
                return glo, ghi

            issue_gathers(0, 0)
            if cfg.n_groups > 1:
                issue_gathers(0, 1)

            iota_t = cpool.tile([P, K_u * P], f16)
            nc.sync.dma_start(out=iota_t[:], in_=iota_wide[:])
            ones_t = cpool.tile([1, P], f16)
            nc.sync.dma_start(out=ones_t[:], in_=ones_row[:])
            linb_t = cpool.tile([1, P], f16)
            nc.sync.dma_start(out=linb_t[:], in_=lin_b_row[:])
            linWT_t = cpool.tile([P, P], f16)
            nc.sync.dma_start(out=linWT_t[:], in_=linWT_d[:])
            WT_t, bt_t = [], []
            for i in range(A):
                wt = cpool.tile([P, P], f16, tag=f"WT{i}")
                nc.sync.dma_start(out=wt[:], in_=WT_d[i, :, :])
                WT_t.append(wt)
                bt = cpool.tile([1, P], f16, tag=f"bt{i}")
                nc.sync.dma_start(out=bt[:], in_=b_rows[i, :, :])
                bt_t.append(bt)
            colloc_t, wchunk_t = [], []
            for i in range(A):
                ct = mpool.tile([P, CH], f16, tag=f"colloc{i}")
                nc.sync.dma_start(out=ct[:], in_=colloc_d[i, :, :])
                colloc_t.append(ct)
                wt = mpool.tile([P, CH], f16, tag=f"wchunk{i}")
                nc.sync.dma_start(out=wt[:], in_=wchunk_d[i, :, :])
                wchunk_t.append(wt)

            # ---- hs0 = relu(x_own @ lin_w.T + lin_b) ----
            for t in range(TPC):
                xo = xpool.tile([P, P], f16, tag="xo")
                nc.sync.dma_start(out=xo[:], in_=xT_own[:, t * P:(t + 1) * P])
                ps = psb.tile([P, P], f32, tag="ps0")
                nc.tensor.matmul(out=ps[:], lhsT=xo[:], rhs=linWT_t[:],
                                 start=True, stop=False)
                nc.tensor.matmul(out=ps[:], lhsT=ones_t[:], rhs=linb_t[:],
                                 start=False, stop=True)
                ob = opool.tile([P, P], f16, tag="ob")
                nc.scalar.activation(out=ob[:], in_=ps[:], func=Act.Relu)
                nc.sync.dma_start(out=hs0[t * P:(t + 1) * P, :], in_=ob[:])

            # ---- scatter phase ----
            for i in range(A):
                for g in range(cfg.n_groups):
                    tg = cfg.group_tiles[g]
                    t0 = cfg.group_starts[g]
                    glo, ghi = issue_gathers(i, g)
                    # prefetch next group's gathers (keeps Pool busy)
                    ni, ng = (i, g + 1) if g + 1 < cfg.n_groups else (i + 1, 0)
                    if ni < A:
                        issue_gathers(ni, ng)
                    for tl in range(tg):
                        tt = t0 + tl
                        psT = psa.tile([P, P], f32, tag="psT")
                        gc0 = tt * K_u
                        bwt = bwpool.tile([P, K_u * P], f16, tag="bw")
                        csl = colloc_t[i][:, gc0:gc0 + K_u].rearrange(
                            "p (k o) -> p k o", o=1).to_broadcast([P, K_u, P])
                        wsl = wchunk_t[i][:, gc0:gc0 + K_u].rearrange(
                            "p (k o) -> p k o", o=1).to_broadcast([P, K_u, P])
                        bw3 = bwt[:].rearrange("p (k c) -> p k c", c=P)
                        io3 = iota_t[:].rearrange("p (k c) -> p k c", c=P)
                        nc.vector.tensor_tensor(out=bw3, in0=io3, in1=csl,
                                                op=Alu.is_equal)
                        nc.vector.tensor_tensor(out=bw3, in0=bw3, in1=wsl,
                                                op=Alu.mult)
                        for k in range(K_u):
                            if k < K_lo:
                                src = glo[:, tl * K_lo + k, :]
                            else:
                                src = ghi[:, tl * K_hi + (k - K_lo), :]
                            nc.tensor.matmul(out=psT[:],
                                             lhsT=src,
                                             rhs=bwt[:, k * P:(k + 1) * P],
                                             start=(k == 0), stop=(k == K_u - 1))
                        sT = spool.tile([P, P], f16, tag="sT")
                        nc.scalar.activation(out=sT[:], in_=psT[:], func=Act.Copy)
                        ps2 = psb.tile([P, P], f32, tag="ps2")
                        nc.tensor.matmul(out=ps2[:], lhsT=WT_t[i][:], rhs=sT[:],
                                         start=True, stop=False)
                        nc.tensor.matmul(out=ps2[:], lhsT=bt_t[i][:], rhs=ones_t[:],
                                         start=False, stop=True)
                        ot = opool.tile([P, P], f16, tag="ot")
                        nc.scalar.activation(out=ot[:], in_=ps2[:], func=Act.Relu)
                        nc.sync.dma_start(out=outsT[i][:, tt * P:(tt + 1) * P],
                                          in_=ot[:])
    nc.finalize()
    return nc


def _assemble(cfg, results):
    N, A = cfg.N, cfg.A
    hs = []
    h0 = np.concatenate([results[k]["hs0"] for k in range(CORES)], axis=0)[:N]
    hs.append(h0.astype(np.float32))
    inv = cfg.colmap[:N]
    for i in range(A):
        o_full = np.concatenate(
            [results[k][f"outT{i}"] for k in range(CORES)], axis=1)  # [P, N2]
        hs.append(np.ascontiguousarray(o_full[:, inv].T).astype(np.float32))
    return tuple(hs)


def kernel(x, edge_index, edge_attr, lin_w, lin_b, conv_w, conv_b):
    global LAST_RESULTS
    x = np.asarray(x, np.float32)
    edge_index = np.asarray(edge_index)
    edge_attr = np.asarray(edge_attr, np.float32)
    lin_w = np.asarray(lin_w, np.float32)
    lin_b = np.asarray(lin_b, np.float32)
    conv_w = np.asarray(conv_w, np.float32)
    conv_b = np.asarray(conv_b, np.float32)

    N, D = x.shape
    A, _, E = edge_index.shape
    assert D == P
    cfg = _make_cfg(N, E, A)
    in_maps = _prep(cfg, x, edge_index, edge_attr, lin_w, lin_b, conv_w, conv_b)
    nc = _build(cfg)

    from concourse.bass_utils import run_bass_kernel_spmd
    res = run_bass_kernel_spmd(nc, in_maps, list(range(CORES)), trace=TRACE)
    LAST_RESULTS = res
    return _assemble(cfg, res.results)


def run_sim(x, edge_index, edge_attr, lin_w, lin_b, conv_w, conv_b, cores=None):
    from concourse import bass_interp
    x = np.asarray(x, np.float32)
    edge_index = np.asarray(edge_index)
    edge_attr = np.asarray(edge_attr, np.float32)
    N, D = x.shape
    A, _, E = edge_index.shape
    cfg = _make_cfg(N, E, A)
    in_maps = _prep(cfg, x, edge_index, edge_attr,
                    np.asarray(lin_w, np.float32), np.asarray(lin_b, np.float32),
                    np.asarray(conv_w, np.float32), np.asarray(conv_b, np.float32))
    results = []
    for k in (range(CORES) if cores is None else cores):
        nc = _build(cfg)
        sim = bass_interp.CoreSim(nc, core_id=0)
        sim.assign_tensors(in_maps[k])
        sim.simulate()
        results.append({name: sim.tensor(name).copy()
                        for name in ["hs0"] + [f"outT{i}" for i in range(A)]})
    if cores is not None:
        return cfg, results
    return _assemble(cfg, results)


# revision 5
# speedup vs baseline: 1.0122x; 1.0122x over previous
"""GCN cascade layer (3 parallel GCNConv + 1 linear head) on 8 Trainium2 cores.

Reformulation (per edge set i):
    norm_e = dinv[row_e] * w_e * dinv[col_e]        (host, f32)
    s[c]   = sum_{e->c} norm_e * x[row_e]            (device: gather + one-hot matmul)
    out[c] = relu(W_i s[c] + b_i)                    (device: per-tile matmul)

W commutes with the edge aggregation, so no dense h = x @ W.T phase is needed.
The scatter accumulates TRANSPOSED: psT[f, c] += src^T @ bw where
src = gathered x rows [slot, f] (lhsT) and bw[slot, c] = one-hot(col) * norm
(rhs), chained over K_u chunks per 128-col tile in PSUM. Then
out^T[g, c] = W_i @ psT (+ b_i via rank-1 matmul), relu, DMA out
feature-major; the host transposes at assembly.

Distribution: output node tiles sharded over 8 cores (TPC=ceil(N/8/128)
tiles each); x replicated (gather source in DRAM, fp16, split into two
int16-addressable overlapping windows lo=[0,32768) / hi=[N2-32768,N2));
flexible rows are assigned lo/hi per (core,set,tile) to balance chunk counts.
"""

import sys

sys.path.insert(0, "/opt/trn_rl_repo")

import math
from dataclasses import dataclass

import numpy as np

import concourse.bass as bass
import concourse.bacc as bacc
import concourse.mybir as mybir
from concourse import tile

P = 128
CORES = 8
GATHER_GROUP = 7   # node tiles per dma_gather call
PAD_COL = 999.0    # colloc sentinel for padded slots (never equals iota)

f16 = mybir.dt.float16
f32 = mybir.dt.float32
i16 = mybir.dt.int16

LAST_RESULTS = None
TRACE = False


@dataclass
class Cfg:
    N: int
    E: int
    A: int
    TPC: int
    NT: int
    N2: int
    OWN: int
    LO_CAP: int      # rows covered by the lo window [0, LO_CAP)
    HI0: int         # hi window start (rows [HI0, N2))
    K_lo: int = 0
    K_hi: int = 0
    K_u: int = 0
    n_groups: int = 0
    group_tiles: tuple = ()
    group_starts: tuple = ()
    colmap: object = None


def _make_cfg(N, E, A):
    TPC = math.ceil(N / (CORES * P))
    NT = CORES * TPC
    N2 = NT * P
    LO_CAP = min(32768, N2)
    HI0 = max(0, N2 - 32768)
    cfg = Cfg(N=N, E=E, A=A, TPC=TPC, NT=NT, N2=N2, OWN=TPC * P,
              LO_CAP=LO_CAP, HI0=HI0)
    g = min(GATHER_GROUP, TPC)
    cfg.group_tiles = tuple(
        min(g, TPC - i * g) for i in range(math.ceil(TPC / g)))
    cfg.n_groups = len(cfg.group_tiles)
    starts = [0]
    for t in cfg.group_tiles[:-1]:
        starts.append(starts[-1] + t)
    cfg.group_starts = tuple(starts)
    return cfg


def _prep(cfg, x, edge_index, edge_attr, lin_w, lin_b, conv_w, conv_b):
    A, N, E, N2 = cfg.A, cfg.N, cfg.E, cfg.N2
    TPC, OWN = cfg.TPC, cfg.OWN

    row = edge_index[:, 0, :].astype(np.int64)    # [A, E]
    col0 = edge_index[:, 1, :].astype(np.int64)
    w = edge_attr.astype(np.float64)

    # ---- normalization (host, original col ids) ----
    norm = np.empty((A, E), np.float32)
    for i in range(A):
        deg = np.bincount(col0[i], weights=w[i], minlength=N2)
        dinv = np.where(deg > 0, 1.0 / np.sqrt(np.maximum(deg, 1e-30)), 0.0)
        norm[i] = (dinv[row[i]] * w[i] * dinv[col0[i]]).astype(np.float32)

    # ---- column rebalance: permute cols across tiles so that every
    # (tile, set) edge count fits the minimal chunk budget. colmap[c] = new id.
    import time as _time
    NTb = cfg.NT
    d = np.zeros((A, N2), np.int64)
    for i in range(A):
        d[i] = np.bincount(col0[i], minlength=N2)
    dtot = d.sum(0)
    order_c = np.argsort(-dtot, kind="stable")
    # load-feedback snake deal
    loads = np.zeros((NTb, A), np.float64)
    bin_of = np.empty(N2, np.int64)
    for r in range(N2 // NTb):
        seg = order_c[r * NTb:(r + 1) * NTb]
        border = np.argsort(loads.max(1) + 0.25 * loads.sum(1), kind="stable")
        bin_of[seg] = border
        loads[border] += d[:, seg].T
    pl = np.zeros((NTb, A), np.int64)
    for i in range(A):
        pl[:, i] = np.bincount(bin_of, weights=d[i], minlength=NTb).astype(np.int64)
    # swap-repair toward cap = next lower multiple of P when within reach
    CAP = max(P, int(math.ceil(max(pl.max(), 1) / P) - 1) * P)
    if pl.mean() < CAP and np.maximum(pl - CAP, 0).sum() > 0:
        cols_of = [list(np.where(bin_of == b)[0]) for b in range(NTb)]
        t0 = _time.time()
        rng_r = np.random.default_rng(12345)
        it = 0
        while np.maximum(pl - CAP, 0).sum() > 0 and it < 5000 \
                and _time.time() - t0 < 45:
            it += 1
            b = int(np.argmax(np.maximum(pl - CAP, 0).sum(1)))
            slack = (CAP - pl).clip(0, None)
            need = np.maximum(pl[b] - CAP, 0)
            cand = np.argsort(-(slack * (need > 0)).sum(1))[:12]
            best = None
            cb_arr = np.array(cols_of[b])
            db = d[:, cb_arr]
            base_b = np.maximum(pl[b] - CAP, 0).sum()
            for b2 in cand:
                if b2 == b:
                    continue
                cb2_arr = np.array(cols_of[b2])
                db2 = d[:, cb2_arr]
                delta = db[:, :, None] - db2[:, None, :]
                nb_new = pl[b][:, None, None] - delta
                n2_new = pl[b2][:, None, None] + delta
                score = (np.maximum(nb_new - CAP, 0).sum(0)
                         + np.maximum(n2_new - CAP, 0).sum(0))
                ix = np.unravel_index(np.argmin(score), score.shape)
                base = base_b + np.maximum(pl[b2] - CAP, 0).sum()
                gain = score[ix] - base
                if gain < 0 and (best is None or gain < best[0]):
                    best = (gain, b2, ix[0], ix[1])
            if best is None:
                b2 = int(rng_r.integers(0, NTb))
                if b2 == b:
                    continue
                best = (0, b2, int(rng_r.integers(0, len(cols_of[b]))),
                        int(rng_r.integers(0, len(cols_of[b2]))))
            _, b2, i1, i2 = best
            c1, c2 = cols_of[b][i1], cols_of[b2][i2]
            cols_of[b][i1], cols_of[b2][i2] = c2, c1
            pl[b] += d[:, c2] - d[:, c1]
            pl[b2] += d[:, c1] - d[:, c2]
        for b in range(NTb):
            bin_of[np.array(cols_of[b])] = b
    # slots within bin: stable order
    colmap = np.empty(N2, np.int64)
    for b in range(NTb):
        cb = np.where(bin_of == b)[0]
        colmap[cb] = b * P + np.arange(len(cb))
    cfg.colmap = colmap
    col = colmap[col0]                                 # [A, E] permuted cols

    # ---- lo/hi stream assignment, balanced per (set, core, tile) ----
    # bucket id per edge: (i*CORES + core)*TPC + tloc
    core = col // OWN                                  # [A, E]
    tloc = (col % OWN) // P
    cint = (col % P).astype(np.float32)
    bucket = (np.arange(A)[:, None] * CORES + core) * TPC + tloc   # [A, E]
    NB = A * CORES * TPC

    use_hi = cfg.N2 > cfg.LO_CAP
    must_hi = row >= cfg.LO_CAP
    flex = (row >= cfg.HI0) & ~must_hi if use_hi else np.zeros_like(must_hi)

    bucket_f = bucket.ravel()
    must_hi_f = must_hi.ravel()
    flex_f = flex.ravel()
    n_tot = np.bincount(bucket_f, minlength=NB)
    n_mhi = np.bincount(bucket_f[must_hi_f], minlength=NB)
    n_flex = np.bincount(bucket_f[flex_f], minlength=NB)
    n_mlo = n_tot - n_mhi - n_flex
    # asymmetric split: cap the hi stream at K_tot//2 chunks so
    # K_lo + K_hi == ceil(max_tot/128) (minimal total chunk count)
    K_tot = max(1, int(math.ceil(n_tot.max() / P)))
    hi_cap = (K_tot // 2) * P if use_hi else 0
    lo_cnt = np.clip(n_tot - hi_cap, n_mlo, n_mlo + n_flex)
    flex_to_lo = lo_cnt - n_mlo

    # stable sort edges by (bucket, class) where class: 0=must_lo,1=flex,2=must_hi
    cls_f = np.zeros(A * E, np.int8)
    cls_f[flex_f] = 1
    cls_f[must_hi_f] = 2
    order = np.lexsort((cls_f, bucket_f))
    # rank within (bucket, class) for flex edges to split them
    key = bucket_f[order] * 4 + cls_f[order]
    change = np.r_[True, key[1:] != key[:-1]]
    starts = np.where(change, np.arange(A * E), 0)
    starts = np.maximum.accumulate(starts)
    rank_in_cls = np.arange(A * E) - starts
    is_lo_sorted = (cls_f[order] == 0) | (
        (cls_f[order] == 1) & (rank_in_cls < flex_to_lo[bucket_f[order]]))
    is_lo = np.empty(A * E, np.bool_)
    is_lo[order] = is_lo_sorted
    is_lo = is_lo.reshape(A, E)

    # ---- per-stream rank within bucket ----
    strm = (~is_lo).astype(np.int8).ravel()            # 0=lo, 1=hi
    order2 = np.lexsort((row.ravel(), strm, bucket_f))  # sort by row for locality
    key2 = bucket_f[order2] * 2 + strm[order2]
    change2 = np.r_[True, key2[1:] != key2[:-1]]
    starts2 = np.where(change2, np.arange(A * E), 0)
    starts2 = np.maximum.accumulate(starts2)
    rank2 = np.arange(A * E) - starts2                 # rank within (bucket, stream)
    rank = np.empty(A * E, np.int64)
    rank[order2] = rank2
    rank = rank.reshape(A, E)

    n_lo_b = np.bincount(bucket_f[~strm.astype(bool)], minlength=NB)
    n_hi_b = np.bincount(bucket_f[strm.astype(bool)], minlength=NB)
    K_lo = max(1, int(math.ceil(n_lo_b.max() / P)))
    K_hi = int(math.ceil(n_hi_b.max() / P)) if use_hi else 0
    cfg.K_lo, cfg.K_hi, cfg.K_u = K_lo, K_hi, K_lo + K_hi

    # ---- metadata arrays ----
    CH = TPC * cfg.K_u
    colloc = np.full((CORES, A, P, CH), PAD_COL, np.float16)
    wchunk = np.zeros((CORES, A, P, CH), np.float16)
    # gather idx streams (per core, set): [16, TPC*K*8] -> tiled to 128 parts
    gl_len = TPC * K_lo * P
    gh_len = max(TPC * K_hi * P, 16)
    gidx_lo = np.zeros((CORES, A, 16, gl_len // 16), np.int16)
    gidx_hi = np.zeros((CORES, A, 16, gh_len // 16), np.int16)

    kk = rank // P
    jj = (rank % P).astype(np.int64)
    lo_m = is_lo
    k_in_tile = np.where(lo_m, kk, K_lo + kk)
    gc = tloc * cfg.K_u + k_in_tile                    # chunk column [A, E]
    ci = np.arange(A)[:, None].repeat(E, 1)
    colloc[core, ci, jj, gc] = cint
    wchunk[core, ci, jj, gc] = norm

    gi = np.where(lo_m, row, row - cfg.HI0).astype(np.int64)
    Kstr = np.where(lo_m, K_lo, K_hi)
    q = tloc * (Kstr * P) + rank                       # stream position
    lo_sel = lo_m
    gidx_lo[core[lo_sel], ci[lo_sel], q[lo_sel] % 16, q[lo_sel] // 16] = \
        gi[lo_sel].astype(np.int16)
    if cfg.K_hi > 0:
        hi_sel = ~lo_m
        gidx_hi[core[hi_sel], ci[hi_sel], q[hi_sel] % 16, q[hi_sel] // 16] = \
            gi[hi_sel].astype(np.int16)

    # ---- dense inputs ----
    xpad = np.zeros((N2 + 8, P), np.float32)
    xpad[:N] = x
    xrows = xpad.astype(np.float16)
    xlo = np.ascontiguousarray(xrows[:cfg.LO_CAP])
    xhi = np.ascontiguousarray(xrows[cfg.HI0:cfg.HI0 + min(32768, N2)])
    xT = np.ascontiguousarray(xrows[:N2].T)            # [P, N2] for hs0

    WT = np.ascontiguousarray(conv_w.transpose(0, 2, 1)).astype(np.float16)
    b_rows = conv_b.reshape(A, 1, P).astype(np.float16)
    linWT = np.ascontiguousarray(lin_w.T).astype(np.float16)
    lin_b_row = lin_b.reshape(1, P).astype(np.float16)
    ones_row = np.ones((1, P), np.float16)
    iota_wide = np.tile(np.arange(P, dtype=np.float16), (P, cfg.K_u))

    in_maps = []
    for k in range(CORES):
        m = dict(
            xlo=xlo, xhi=xhi,
            xT_own=np.ascontiguousarray(xT[:, k * OWN:(k + 1) * OWN]),
            WT=WT, b_rows=b_rows, linWT=linWT, lin_b_row=lin_b_row,
            ones_row=ones_row, iota_wide=iota_wide,
            colloc=colloc[k], wchunk=wchunk[k],
            gidx_lo=np.tile(gidx_lo[k], (1, 8, 1)),
        )
        if cfg.K_hi > 0:
            m["gidx_hi"] = np.tile(gidx_hi[k], (1, 8, 1))
        in_maps.append(m)
    return in_maps


def _build(cfg):
    nc = bacc.Bacc()
    A, TPC = cfg.A, cfg.TPC
    K_lo, K_hi, K_u = cfg.K_lo, cfg.K_hi, cfg.K_u
    CH = TPC * K_u
    Alu = mybir.AluOpType
    Act = mybir.ActivationFunctionType

    xlo_d = nc.dram_tensor("xlo", [cfg.LO_CAP, P], f16, kind="ExternalInput")
    xhi_d = nc.dram_tensor("xhi", [min(32768, cfg.N2), P], f16,
                           kind="ExternalInput")
    xT_own = nc.dram_tensor("xT_own", [P, cfg.OWN], f16, kind="ExternalInput")
    WT_d = nc.dram_tensor("WT", [A, P, P], f16, kind="ExternalInput")
    b_rows = nc.dram_tensor("b_rows", [A, 1, P], f16, kind="ExternalInput")
    linWT_d = nc.dram_tensor("linWT", [P, P], f16, kind="ExternalInput")
    lin_b_row = nc.dram_tensor("lin_b_row", [1, P], f16, kind="ExternalInput")
    ones_row = nc.dram_tensor("ones_row", [1, P], f16, kind="ExternalInput")
    iota_wide = nc.dram_tensor("iota_wide", [P, K_u * P], f16, kind="ExternalInput")
    colloc_d = nc.dram_tensor("colloc", [A, P, CH], f16, kind="ExternalInput")
    wchunk_d = nc.dram_tensor("wchunk", [A, P, CH], f16, kind="ExternalInput")
    gl_len = TPC * K_lo * P
    gh_len = max(TPC * K_hi * P, 16)
    gidx_lo_d = nc.dram_tensor("gidx_lo", [A, P, gl_len // 16], i16,
                               kind="ExternalInput")
    gidx_hi_d = (nc.dram_tensor("gidx_hi", [A, P, gh_len // 16], i16,
                                kind="ExternalInput") if K_hi > 0 else None)

    hs0 = nc.dram_tensor("hs0", [cfg.OWN, P], f16, kind="ExternalOutput")
    outsT = [nc.dram_tensor(f"outT{i}", [P, cfg.OWN], f16, kind="ExternalOutput")
             for i in range(A)]

    GT = min(GATHER_GROUP, TPC)

    with tile.TileContext(nc) as tc:
        with (
            tc.tile_pool(name="const", bufs=1) as cpool,
            tc.tile_pool(name="meta", bufs=1) as mpool,
            tc.tile_pool(name="xw", bufs=3) as xpool,
            tc.tile_pool(name="glo", bufs=3) as glo_pool,
            tc.tile_pool(name="ghi", bufs=3) as ghi_pool,
            tc.tile_pool(name="gix", bufs=4) as gix_pool,
            tc.tile_pool(name="bw", bufs=6) as bwpool,
            tc.tile_pool(name="st", bufs=4) as spool,
            tc.tile_pool(name="outst", bufs=4) as opool,
            tc.tile_pool(name="psa", bufs=4, space="PSUM") as psa,
            tc.tile_pool(name="psb", bufs=2, space="PSUM") as psb,
        ):
            # ---- gather issue helper (idempotent per (set, group)) ----
            gather_bufs = {}

            def issue_gathers(i, g):
                if (i, g) in gather_bufs:
                    return gather_bufs[(i, g)]
                tg = cfg.group_tiles[g]
                t0 = cfg.group_starts[g]
                L = tg * K_lo * P
                glo = glo_pool.tile([P, GT * K_lo, P], f16, tag="glo")
                gixt = gix_pool.tile([P, GT * K_lo * 8], i16, tag="gixlo")
                off = t0 * K_lo * 8
                nc.sync.dma_start(out=gixt[:, :L // 16],
                                  in_=gidx_lo_d[i, :, off:off + L // 16])
                nc.gpsimd.dma_gather(
                    out_ap=glo[:, :tg * K_lo, :], in_ap=xlo_d[:, :],
                    idxs_ap=gixt[:, :L // 16],
                    num_idxs=L, num_idxs_reg=L, elem_size=P,
                    single_packet=False)
                ghi = None
                if K_hi > 0:
                    Lh = tg * K_hi * P
                    ghi = ghi_pool.tile([P, GT * K_hi, P], f16, tag="ghi")
                    gixh = gix_pool.tile([P, GT * K_hi * 8], i16, tag="gixhi")
                    offh = t0 * K_hi * 8
                    nc.sync.dma_start(out=gixh[:, :Lh // 16],
                                      in_=gidx_hi_d[i, :, offh:offh + Lh // 16])
                    nc.gpsimd.dma_gather(
                        out_ap=ghi[:, :tg * K_hi, :], in_ap=xhi_d[:, :],
                        idxs_ap=gixh[:, :Lh // 16],
                        num_idxs=Lh, num_idxs_reg=Lh, elem_size=P,
                        single_packet=False)
                gather_bufs[(i, g)] = (glo, ghi)
                return glo, ghi

            issue_gathers(0, 0)
            if cfg.n_groups > 1:
                issue_gathers(0, 1)

            iota_t = cpool.tile([P, K_u * P], f16)
            nc.sync.dma_start(out=iota_t[:], in_=iota_wide[:])
            ones_t = cpool.tile([1, P], f16)
            nc.sync.dma_start(out=ones_t[:], in_=ones_row[:])
            linb_t = cpool.tile([1, P], f16)
            nc.sync.dma_start(out=linb_t[:], in_=lin_b_row[:])
            linWT_t = cpool.tile([P, P], f16)
            nc.sync.dma_start(out=linWT_t[:], in_=linWT_d[:])
            WT_t, bt_t = [], []
            for i in range(A):
                wt = cpool.tile([P, P], f16, tag=f"WT{i}")
                nc.sync.dma_start(out=wt[:], in_=WT_d[i, :, :])
                WT_t.append(wt)
                bt = cpool.tile([1, P], f16, tag=f"bt{i}")
                nc.sync.dma_start(out=bt[:], in_=b_rows[i, :, :])
                bt_t.append(bt)
            colloc_t, wchunk_t = [], []
            for i in range(A):
                ct = mpool.tile([P, CH], f16, tag=f"colloc{i}")
                nc.sync.dma_start(out=ct[:], in_=colloc_d[i, :, :])
                colloc_t.append(ct)
                wt = mpool.tile([P, CH], f16, tag=f"wchunk{i}")
                nc.sync.dma_start(out=wt[:], in_=wchunk_d[i, :, :])
                wchunk_t.append(wt)

            # ---- hs0 = relu(x_own @ lin_w.T + lin_b) ----
            for t in range(TPC):
                xo = xpool.tile([P, P], f16, tag="xo")
                nc.sync.dma_start(out=xo[:], in_=xT_own[:, t * P:(t + 1) * P])
                ps = psb.tile([P, P], f32, tag="ps0")
                nc.tensor.matmul(out=ps[:], lhsT=xo[:], rhs=linWT_t[:],
                                 start=True, stop=False)
                nc.tensor.matmul(out=ps[:], lhsT=ones_t[:], rhs=linb_t[:],
                                 start=False, stop=True)
                ob = opool.tile([P, P], f16, tag="ob")
                nc.scalar.activation(out=ob[:], in_=ps[:], func=Act.Relu)
                nc.sync.dma_start(out=hs0[t * P:(t + 1) * P, :], in_=ob[:])

            # ---- scatter phase ----
            for i in range(A):
                for g in range(cfg.n_groups):
                    tg = cfg.group_tiles[g]
                    t0 = cfg.group_starts[g]
                    glo, ghi = issue_gathers(i, g)
                    # prefetch next group's gathers (keeps Pool busy)
                    ni, ng = (i, g + 1) if g + 1 < cfg.n_groups else (i + 1, 0)
                    if ni < A:
                        issue_gathers(ni, ng)
                    for tl in range(tg):
                        tt = t0 + tl
                        psT = psa.tile([P, P], f32, tag="psT")
                        gc0 = tt * K_u
                        bwt = bwpool.tile([P, K_u * P], f16, tag="bw")
                        csl = colloc_t[i][:, gc0:gc0 + K_u].rearrange(
                            "p (k o) -> p k o", o=1).to_broadcast([P, K_u, P])
                        wsl = wchunk_t[i][:, gc0:gc0 + K_u].rearrange(
                            "p (k o) -> p k o", o=1).to_broadcast([P, K_u, P])
                        bw3 = bwt[:].rearrange("p (k c) -> p k c", c=P)
                        io3 = iota_t[:].rearrange("p (k c) -> p k c", c=P)
                        nc.vector.tensor_tensor(out=bw3, in0=io3, in1=csl,
                                                op=Alu.is_equal)
                        nc.vector.tensor_tensor(out=bw3, in0=bw3, in1=wsl,
                                                op=Alu.mult)
                        for k in range(K_u):
                            if k < K_lo:
                                src = glo[:, tl * K_lo + k, :]
                            else:
                                src = ghi[:, tl * K_hi + (k - K_lo), :]
                            nc.tensor.matmul(out=psT[:],
                                             lhsT=src,
                                             rhs=bwt[:, k * P:(k + 1) * P],
                                             start=(k == 0), stop=(k == K_u - 1))
                        sT = spool.tile([P, P], f16, tag="sT")
                        nc.scalar.activation(out=sT[:], in_=psT[:], func=Act.Copy)
                        ps2 = psb.tile([P, P], f32, tag="ps2")
                        nc.tensor.matmul(out=ps2[:], lhsT=WT_t[i][:], rhs=sT[:],
                                         start=True, stop=False)
                        nc.tensor.matmul(out=ps2[:], lhsT=bt_t[i][:], rhs=ones_t[:],
                                         start=False, stop=True)
                        ot = opool.tile([P, P], f16, tag="ot")
                        nc.scalar.activation(out=ot[:], in_=ps2[:], func=Act.Relu)
                        nc.sync.dma_start(out=outsT[i][:, tt * P:(tt + 1) * P],
                                          in_=ot[:])
    nc.finalize()
    return nc


def _assemble(cfg, results):
    N, A = cfg.N, cfg.A
    hs = []
    h0 = np.concatenate([results[k]["hs0"] for k in range(CORES)], axis=0)[:N]
    hs.append(h0.astype(np.float32))
    inv = cfg.colmap[:N]
    for i in range(A):
        o_full = np.concatenate(
            [results[k][f"outT{i}"] for k in range(CORES)], axis=1)  # [P, N2]
        hs.append(np.ascontiguousarray(o_full[:, inv].T).astype(np.float32))
    return tuple(hs)


def kernel(x, edge_index, edge_attr, lin_w, lin_b, conv_w, conv_b):
    global LAST_RESULTS
    x = np.asarray(x, np.float32)
    edge_index = np.asarray(edge_index)
    edge_attr = np.asarray(edge_attr, np.float32)
    lin_w = np.asarray(lin_w, np.float32)
    lin_b = np.asarray(lin_b, np.float32)
    conv_w = np.asarray(conv_w, np.float32)
    conv_b = np.asarray(conv_b, np.float32)

    N, D = x.shape
    A, _, E = edge_index.shape
    assert D == P
    cfg = _make_cfg(N, E, A)
    in_maps = _prep(cfg, x, edge_index, edge_attr, lin_w, lin_b, conv_w, conv_b)
    nc = _build(cfg)

    from concourse.bass_utils import run_bass_kernel_spmd
    res = run_bass_kernel_spmd(nc, in_maps, list(range(CORES)), trace=TRACE)
    LAST_RESULTS = res
    return _assemble(cfg, res.results)


def run_sim(x, edge_index, edge_attr, lin_w, lin_b, conv_w, conv_b, cores=None):
    from concourse import bass_interp
    x = np.asarray(x, np.float32)
    edge_index = np.asarray(edge_index)
    edge_attr = np.asarray(edge_attr, np.float32)
    N, D = x.shape
    A, _, E = edge_index.shape
    cfg = _make_cfg(N, E, A)
    in_maps = _prep(cfg, x, edge_index, edge_attr,
                    np.asarray(lin_w, np.float32), np.asarray(lin_b, np.float32),
                    np.asarray(conv_w, np.float32), np.asarray(conv_b, np.float32))
    results = []
    for k in (range(CORES) if cores is None else cores):
        nc = _build(cfg)
        sim = bass_interp.CoreSim(nc, core_id=0)
        sim.assign_tensors(in_maps[k])
        sim.simulate()
        results.append({name: sim.tensor(name).copy()
                        for name in ["hs0"] + [f"outT{i}" for i in range(A)]})
    if cores is not None:
        return cfg, results
    return _assemble(cfg, results)
